# revision 15
# baseline (speedup 1.0000x reference)
"""Trainium2 Bass kernel for nn_LoRALinear1d.

Math: out[b] = (W_main + a_in[b] @ a_out[b]) @ x[b] + b_main
  with a_in[b] = reshape(W_ain @ g[b], [CIN, R]),
       a_out[b] = reshape(W_aout @ g[b], [R, COUT]).

Sharding: data-parallel over batch B=8, one batch per NeuronCore (8 cores).
All adapter math is folded on-device into an effective transposed weight
W_effT[i, o] = W_main[o, i] + (a_in @ a_out)[i, o], then a tiled
[256,256] x [256, L] matmul runs over L with the bias add fused into the
PSUM->SBUF eviction.

Memory-bound problem, so x and out travel as bf16 (host converts both
ways): 16 MB read + 16 MB write per core instead of 64 MB round trip in
fp32. The host also pre-transposes the small weights (pure marshalling)
so the device fold needs no PE transposes, and pre-permutes W_ain's
columns so both adapter rows land as free-dim slices of partition 0's
a_flat row - from there the rank-2 LoRA outer product is two K=1
accumulating matmuls with no partition shuffles at all. Total rel err
~3e-3 from the bf16 roundings, far under the 2e-2 gate.

Engine queues (each engine issues its own instruction stream in order;
each queue maps to its own DMA descriptor ring, so streams don't block
each other):
  Sync    - adapter weights + g first (they head the fold's dependency
            chain), then the 16 big x loads into a 16-buffer pool so the
            read stream never waits on compute
  Scalar  - wmainT/bias loads, half the PSUM evictions (bias via
            activation)
  Vector  - other half of evictions (tensor_scalar add), small fold copies
  Tensor  - adapter matvecs, rank-2 LoRA product, all main matmuls
  GpSimd  - output stores only (store triggers wait on both eviction
            engines; on a dedicated queue they can't stall anyone)
"""

from contextlib import ExitStack

import ml_dtypes
import numpy as np

import concourse.bacc as bacc
import concourse.mybir as mybir
import concourse.tile as tile
from concourse.bass_utils import run_bass_kernel_spmd

B, CIN, COUT, CINFO, R, L = 8, 256, 256, 256, 2, 32768
P = 128
LC = 2048           # L elements per SBUF tile
F32 = mybir.dt.float32
BF16 = mybir.dt.bfloat16
BF16_NP = ml_dtypes.bfloat16


def _build():
    nc = bacc.Bacc("TRN2", target_bir_lowering=False, debug=False)
    x = nc.dram_tensor("x", [CIN, L], BF16, kind="ExternalInput").ap()
    g = nc.dram_tensor("g", [CINFO], BF16, kind="ExternalInput").ap()
    # wmainT[i, o] = W_main[o, i]
    wmainT = nc.dram_tensor("wmainT", [CIN, COUT], BF16, kind="ExternalInput").ap()
    bmain = nc.dram_tensor("bmain", [COUT], F32, kind="ExternalInput").ap()
    # wainT[c, r*CIN + i] = W_ain[i*R + r, c];  waoutT[c, r*COUT + o] = W_aout[r*COUT + o, c]
    wainT = nc.dram_tensor("wainT", [CINFO, CIN * R], BF16, kind="ExternalInput").ap()
    waoutT = nc.dram_tensor("waoutT", [CINFO, COUT * R], BF16, kind="ExternalInput").ap()
    out = nc.dram_tensor("out", [COUT, L], BF16, kind="ExternalOutput").ap()

    x_v = x.rearrange("(t p) l -> p t l", p=P)
    out_v = out.rearrange("(t p) l -> p t l", p=P)
    NCH = L // LC

    with tile.TileContext(nc) as tc, ExitStack() as ctx:
        consts = ctx.enter_context(tc.tile_pool(name="consts", bufs=1))
        xpool = ctx.enter_context(tc.tile_pool(name="xp", bufs=12))
        opool = ctx.enter_context(tc.tile_pool(name="op", bufs=8))
        pre = ctx.enter_context(tc.tile_pool(name="pre", bufs=1))

        # fold inputs lead the Sync ring (512 KB: lands at line rate in ~2us),
        # then the 16 x loads fire into a 12-buffer pool (the last 4 wait on
        # early chunks, by which time the read stream shares with stores
        # anyway); the freed SBUF buys 8 output buffers so the store backlog
        # that accumulates mid-kernel never stalls eviction or the PE
        g_sb = consts.tile([P, CINFO // P], BF16)  # g[c] at [c%128, c//128]
        nc.sync.dma_start(g_sb[:], g.rearrange("(h p) -> p h", p=P))
        wT_ain = pre.tile([P, 2, 512], BF16, name="wT_ain")
        nc.sync.dma_start(wT_ain[:], wainT.rearrange("(h p) n -> p h n", p=P))
        wT_aout = pre.tile([P, 2, 512], BF16, name="wT_aout")
        nc.sync.dma_start(wT_aout[:], waoutT.rearrange("(h p) n -> p h n", p=P))

        xts = []
        for ci in range(NCH):
            x_t = xpool.tile([P, CIN // P, LC], BF16, name="x_t")
            nc.sync.dma_start(x_t[:], x_v[:, :, ci * LC:(ci + 1) * LC])
            xts.append(x_t)

        # wmainT/bias ride the otherwise-empty Scalar ring; they are only
        # needed at the very end of the fold
        b_sb = consts.tile([P, COUT // P], F32)    # bias per o-tile column
        nc.scalar.dma_start(b_sb[:], bmain.rearrange("(h p) -> p h", p=P))
        wmT_bf = pre.tile([P, CIN // P, COUT], BF16)
        nc.scalar.dma_start(wmT_bf[:], wmainT.rearrange("(t p) o -> p t o", p=P))
        wmT = pre.tile([P, CIN // P, COUT], F32)
        nc.vector.tensor_copy(wmT[:], wmT_bf[:])

        # W_effT[i_tile][i, o] (i on partitions)
        weffT = [consts.tile([P, COUT], BF16, name=f"weffT{i}") for i in range(CIN // P)]

        with tc.tile_pool(name="prepsum", bufs=1, space="PSUM") as prepsum:
            # adapter rows: a_flat[n] = sum_c W_zT[c, n] g[c], K=c on
            # partitions; partition 0 holds the full 512-wide a_flat row
            arows = {}
            for wT, nm in ((wT_ain, "ain"), (wT_aout, "aout")):
                a_ps = prepsum.tile([1, 512], F32, name=f"aps_{nm}", tag=f"aps_{nm}")
                for h in range(2):
                    nc.tensor.matmul(
                        a_ps[:], g_sb[:, h:h + 1], wT[:, h, :],
                        start=(h == 0), stop=(h == 1),
                    )
                a_row = pre.tile([1, 512], F32, name=f"arow_{nm}", tag=f"arow_{nm}")
                nc.vector.tensor_copy(a_row[:], a_ps[:])
                arows[nm] = a_row

            # W_effT = W_mainT + a_in @ a_out as two accumulating K=1 rank-1
            # updates; both r-blocks are free-dim slices of partition 0's row
            for it in range(2):
                lora_ps = prepsum.tile([P, COUT], F32, name=f"lorap{it}", tag="lorap")
                for r in range(R):
                    nc.tensor.matmul(
                        lora_ps[:],
                        arows["ain"][:, r * 256 + it * P:r * 256 + (it + 1) * P],
                        arows["aout"][:, r * 256:(r + 1) * 256],
                        start=(r == 0), stop=(r == R - 1),
                    )
                nc.vector.tensor_add(weffT[it][:], wmT[:, it, :], lora_ps[:])

        # main loop over L.  Per chunk: 16 matmuls into 2-bank PSUM tiles,
        # 4 evictions (split ScalarE/VectorE) converting fp32 PSUM -> bf16,
        # one 1 MB store issued from the GpSimd queue.
        pspool = ctx.enter_context(tc.tile_pool(name="psp", bufs=4, space="PSUM"))
        EV = 1024  # eviction width: 2 PSUM banks
        for ci in range(NCH):
            xmm = xts[ci]
            o_t = opool.tile([P, COUT // P, LC], BF16, name="o_t")
            for m in range(2):
                for h in range(LC // EV):
                    ps = pspool.tile([P, EV], F32, name="ps")
                    for k in range(2):
                        for s in range(EV // 512):
                            nc.tensor.matmul(
                                ps[:, s * 512:(s + 1) * 512],
                                weffT[k][:, m * P:(m + 1) * P],
                                xmm[:, k, h * EV + s * 512:h * EV + (s + 1) * 512],
                                start=(k == 0), stop=(k == 1),
                            )
                    osl = o_t[:, m, h * EV:(h + 1) * EV]
                    if m == 0:
                        nc.scalar.activation(
                            osl, ps[:],
                            mybir.ActivationFunctionType.Identity,
                            bias=b_sb[:, m:m + 1],
                        )
                    else:
                        nc.vector.tensor_scalar_add(osl, ps[:], b_sb[:, m:m + 1])
            nc.gpsimd.dma_start(out_v[:, :, ci * LC:(ci + 1) * LC], o_t[:])

    nc.compile()
    return nc


_NC = None
LAST_RESULTS = None  # BassKernelResults from the most recent run


def _in_maps(x, g_out, W_main, b_main, W_ain, W_aout):
    wmainT = np.ascontiguousarray(W_main.T, dtype=np.float32).astype(BF16_NP)
    bmain = np.ascontiguousarray(b_main, dtype=np.float32)
    # reorder so (W_zT @ g) lands as [r, 256] in the PE output row
    wainT = np.ascontiguousarray(
        np.asarray(W_ain, dtype=np.float32)
        .reshape(CIN, R, CINFO).transpose(2, 1, 0).reshape(CINFO, R * CIN)
    ).astype(BF16_NP)
    waoutT = np.ascontiguousarray(W_aout.T, dtype=np.float32).astype(BF16_NP)
    maps = []
    for b in range(B):
        maps.append({
            "x": np.ascontiguousarray(x[b]).astype(BF16_NP),
            "g": np.ascontiguousarray(g_out[b, :, 0], dtype=np.float32).astype(BF16_NP),
            "wmainT": wmainT,
            "bmain": bmain,
            "wainT": wainT,
            "waoutT": waoutT,
        })
    return maps


def kernel(x, g_out, W_main, b_main, W_ain, W_aout, trace=False):
    global _NC, LAST_RESULTS
    if _NC is None:
        _NC = _build()
    maps = _in_maps(x, g_out, W_main, b_main, W_ain, W_aout)
    LAST_RESULTS = run_bass_kernel_spmd(
        _NC, maps, core_ids=list(range(B)), trace=trace
    )
    return np.stack(
        [LAST_RESULTS.results[b]["out"].astype(np.float32) for b in range(B)], axis=0
    )


# revision 17
# speedup vs baseline: 1.0174x; 1.0174x over previous
"""Trainium2 Bass kernel for nn_LoRALinear1d.

Math: out[b] = (W_main + a_in[b] @ a_out[b]) @ x[b] + b_main
  with a_in[b] = reshape(W_ain @ g[b], [CIN, R]),
       a_out[b] = reshape(W_aout @ g[b], [R, COUT]).

Sharding: data-parallel over batch B=8, one batch per NeuronCore (8 cores).
All adapter math is folded on-device into an effective transposed weight
W_effT[i, o] = W_main[o, i] + (a_in @ a_out)[i, o], then a tiled
[256,256] x [256, L] matmul runs over L with the bias add fused into the
PSUM->SBUF eviction.

Memory-bound problem, so x and out travel as bf16 (host converts both
ways): 16 MB read + 16 MB write per core instead of 64 MB round trip in
fp32. The host also pre-transposes the small weights (pure marshalling)
so the device fold needs no PE transposes, and pre-permutes W_ain's
columns so both adapter rows land as free-dim slices of partition 0's
a_flat row - from there the rank-2 LoRA outer product is two K=1
accumulating matmuls with no partition shuffles at all. Total rel err
~3e-3 from the bf16 roundings, far under the 2e-2 gate.

Engine queues (each engine issues its own instruction stream in order;
each queue maps to its own DMA descriptor ring, so streams don't block
each other):
  Sync    - adapter weights + g first (they head the fold's dependency
            chain), then the 16 big x loads into a 16-buffer pool so the
            read stream never waits on compute
  Scalar  - wmainT/bias loads, half the PSUM evictions (bias via
            activation)
  Vector  - other half of evictions (tensor_scalar add), small fold copies
  Tensor  - adapter matvecs, rank-2 LoRA product, all main matmuls
  GpSimd  - output stores only (store triggers wait on both eviction
            engines; on a dedicated queue they can't stall anyone)
"""

from contextlib import ExitStack

import ml_dtypes
import numpy as np

import concourse.bacc as bacc
import concourse.mybir as mybir
import concourse.tile as tile
from concourse.bass_utils import run_bass_kernel_spmd

B, CIN, COUT, CINFO, R, L = 8, 256, 256, 256, 2, 32768
P = 128
LC = 2048           # L elements per SBUF tile
F32 = mybir.dt.float32
BF16 = mybir.dt.bfloat16
BF16_NP = ml_dtypes.bfloat16


def _build():
    nc = bacc.Bacc("TRN2", target_bir_lowering=False, debug=False)
    x = nc.dram_tensor("x", [CIN, L], BF16, kind="ExternalInput").ap()
    g = nc.dram_tensor("g", [CINFO], BF16, kind="ExternalInput").ap()
    # wmainT[i, o] = W_main[o, i]
    wmainT = nc.dram_tensor("wmainT", [CIN, COUT], BF16, kind="ExternalInput").ap()
    bmain = nc.dram_tensor("bmain", [COUT], F32, kind="ExternalInput").ap()
    # wainT[c, r*CIN + i] = W_ain[i*R + r, c];  waoutT[c, r*COUT + o] = W_aout[r*COUT + o, c]
    wainT = nc.dram_tensor("wainT", [CINFO, CIN * R], BF16, kind="ExternalInput").ap()
    waoutT = nc.dram_tensor("waoutT", [CINFO, COUT * R], BF16, kind="ExternalInput").ap()
    out = nc.dram_tensor("out", [COUT, L], BF16, kind="ExternalOutput").ap()

    x_v = x.rearrange("(t p) l -> p t l", p=P)
    out_v = out.rearrange("(t p) l -> p t l", p=P)
    NCH = L // LC

    with tile.TileContext(nc) as tc, ExitStack() as ctx:
        consts = ctx.enter_context(tc.tile_pool(name="consts", bufs=1))
        xpool = ctx.enter_context(tc.tile_pool(name="xp", bufs=12))
        opool = ctx.enter_context(tc.tile_pool(name="op", bufs=8))
        pre = ctx.enter_context(tc.tile_pool(name="pre", bufs=1))

        # fold inputs lead the Sync ring (512 KB: lands at line rate in ~2us),
        # then the 16 x loads fire into a 12-buffer pool (the last 4 wait on
        # early chunks, by which time the read stream shares with stores
        # anyway); the freed SBUF buys 8 output buffers so the store backlog
        # that accumulates mid-kernel never stalls eviction or the PE
        g_sb = consts.tile([P, CINFO // P], BF16)  # g[c] at [c%128, c//128]
        nc.sync.dma_start(g_sb[:], g.rearrange("(h p) -> p h", p=P))
        wT_ain = pre.tile([P, 2, 512], BF16, name="wT_ain")
        nc.sync.dma_start(wT_ain[:], wainT.rearrange("(h p) n -> p h n", p=P))
        wT_aout = pre.tile([P, 2, 512], BF16, name="wT_aout")
        nc.sync.dma_start(wT_aout[:], waoutT.rearrange("(h p) n -> p h n", p=P))

        xts = []
        for ci in range(NCH):
            x_t = xpool.tile([P, CIN // P, LC], BF16, name="x_t")
            nc.sync.dma_start(x_t[:], x_v[:, :, ci * LC:(ci + 1) * LC])
            xts.append(x_t)

        # wmainT/bias ride the otherwise-empty Scalar ring; they are only
        # needed at the very end of the fold
        b_sb = consts.tile([P, COUT // P], F32)    # bias per o-tile column
        nc.scalar.dma_start(b_sb[:], bmain.rearrange("(h p) -> p h", p=P))
        wmT_bf = pre.tile([P, CIN // P, COUT], BF16)
        nc.scalar.dma_start(wmT_bf[:], wmainT.rearrange("(t p) o -> p t o", p=P))

        # W_effT[i_tile][i, o] (i on partitions)
        weffT = [consts.tile([P, COUT], BF16, name=f"weffT{i}") for i in range(CIN // P)]

        with tc.tile_pool(name="prepsum", bufs=1, space="PSUM") as prepsum:
            # adapter rows: a_flat[n] = sum_c W_zT[c, n] g[c], K=c on
            # partitions; partition 0 holds the full 512-wide a_flat row
            arows = {}
            for wT, nm in ((wT_ain, "ain"), (wT_aout, "aout")):
                a_ps = prepsum.tile([1, 512], F32, name=f"aps_{nm}", tag=f"aps_{nm}")
                for h in range(2):
                    nc.tensor.matmul(
                        a_ps[:], g_sb[:, h:h + 1], wT[:, h, :],
                        start=(h == 0), stop=(h == 1),
                    )
                a_row = pre.tile([1, 512], F32, name=f"arow_{nm}", tag=f"arow_{nm}")
                nc.vector.tensor_copy(a_row[:], a_ps[:])
                arows[nm] = a_row

            # upconvert W_mainT after the arow copies so a late wmT DMA can't
            # block them on the Vector queue (it is only needed for the adds)
            wmT = pre.tile([P, CIN // P, COUT], F32)
            nc.vector.tensor_copy(wmT[:], wmT_bf[:])

            # W_effT = W_mainT + a_in @ a_out as two accumulating K=1 rank-1
            # updates; both r-blocks are free-dim slices of partition 0's row
            for it in range(2):
                lora_ps = prepsum.tile([P, COUT], F32, name=f"lorap{it}", tag=f"lorap{it}")
                for r in range(R):
                    nc.tensor.matmul(
                        lora_ps[:],
                        arows["ain"][:, r * 256 + it * P:r * 256 + (it + 1) * P],
                        arows["aout"][:, r * 256:(r + 1) * 256],
                        start=(r == 0), stop=(r == R - 1),
                    )
                nc.vector.tensor_add(weffT[it][:], wmT[:, it, :], lora_ps[:])

        # main loop over L.  Per chunk: 16 matmuls into 2-bank PSUM tiles,
        # 4 evictions (split ScalarE/VectorE) converting fp32 PSUM -> bf16,
        # one 1 MB store issued from the GpSimd queue.
        pspool = ctx.enter_context(tc.tile_pool(name="psp", bufs=4, space="PSUM"))
        EV = 1024  # eviction width: 2 PSUM banks
        for ci in range(NCH):
            xmm = xts[ci]
            o_t = opool.tile([P, COUT // P, LC], BF16, name="o_t")
            for m in range(2):
                for h in range(LC // EV):
                    ps = pspool.tile([P, EV], F32, name="ps")
                    for k in range(2):
                        for s in range(EV // 512):
                            nc.tensor.matmul(
                                ps[:, s * 512:(s + 1) * 512],
                                weffT[k][:, m * P:(m + 1) * P],
                                xmm[:, k, h * EV + s * 512:h * EV + (s + 1) * 512],
                                start=(k == 0), stop=(k == 1),
                            )
                    osl = o_t[:, m, h * EV:(h + 1) * EV]
                    if m == 0:
                        nc.scalar.activation(
                            osl, ps[:],
                            mybir.ActivationFunctionType.Identity,
                            bias=b_sb[:, m:m + 1],
                        )
                    else:
                        nc.vector.tensor_scalar_add(osl, ps[:], b_sb[:, m:m + 1])
            nc.gpsimd.dma_start(out_v[:, :, ci * LC:(ci + 1) * LC], o_t[:])

    nc.compile()
    return nc


_NC = None
LAST_RESULTS = None  # BassKernelResults from the most recent run


def _in_maps(x, g_out, W_main, b_main, W_ain, W_aout):
    wmainT = np.ascontiguousarray(W_main.T, dtype=np.float32).astype(BF16_NP)
    bmain = np.ascontiguousarray(b_main, dtype=np.float32)
    # reorder so (W_zT @ g) lands as [r, 256] in the PE output row
    wainT = np.ascontiguousarray(
        np.asarray(W_ain, dtype=np.float32)
        .reshape(CIN, R, CINFO).transpose(2, 1, 0).reshape(CINFO, R * CIN)
    ).astype(BF16_NP)
    waoutT = np.ascontiguousarray(W_aout.T, dtype=np.float32).astype(BF16_NP)
    maps = []
    for b in range(B):
        maps.append({
            "x": np.ascontiguousarray(x[b]).astype(BF16_NP),
            "g": np.ascontiguousarray(g_out[b, :, 0], dtype=np.float32).astype(BF16_NP),
            "wmainT": wmainT,
            "bmain": bmain,
            "wainT": wainT,
            "waoutT": waoutT,
        })
    return maps


def kernel(x, g_out, W_main, b_main, W_ain, W_aout, trace=False):
    global _NC, LAST_RESULTS
    if _NC is None:
        _NC = _build()
    maps = _in_maps(x, g_out, W_main, b_main, W_ain, W_aout)
    LAST_RESULTS = run_bass_kernel_spmd(
        _NC, maps, core_ids=list(range(B)), trace=trace
    )
    return np.stack(
        [LAST_RESULTS.results[b]["out"].astype(np.float32) for b in range(B)], axis=0
    )
